# revision 1
# baseline (speedup 1.0000x reference)
"""Longformer attention (B=1, S=4096, D=512, H=8, HD=64, window=512, nglobal=64)
on 8 Trainium2 NeuronCores, head-parallel (core c computes head c).

Layout strategy (per core):
  - Host pre-transposes inputs to xT [512, 4096] and pre-rounds matmul operands
    to fp32r (fp32 with 12-bit mantissa) so the PE runs fp32r at full rate.
  - Projections computed transposed: qT/kT [128(d_sw|d_g), 4096] via
    matmul(lhsT=w[f,d], rhs=xT[f,s]).  v produced transposed then PE-transposed
    to natural [s, d] with an appended ones column (row-sum trick).
  - Sliding-window attention in transposed-logits form: per 256-query supertile,
    5-6 key tiles of 128; logits.T [k,q] tiles, exp on ACT (scale=1/8 folded),
    static triangular 0/1 masks multiply 4 of 6 tiles; AV as
    xT' = [v|1].T @ expw.T giving [d+1, q] with softmax denominators in row 64.
  - Global attention (rows < ng) done densely over all 4096 keys.
  - Out-projection natural: matmul(lhsT=xT[d,q], rhs=w_out[d,f]); the softmax
    normalization (1/sum) is applied per-partition during the psum evacuation.
  - Host sums the 8 per-head partial outputs and adds b_out.
"""
import os
import sys
import functools

for _p in ("/opt/trn_rl_repo",):
    if os.path.isdir(_p) and _p not in sys.path:
        sys.path.insert(0, _p)

import numpy as np

import concourse.bass as bass
import concourse.tile as tile
from concourse import bacc, mybir
from concourse.bass_utils import run_bass_kernel_spmd

S = 4096
F = 512          # d_model
HD = 64          # head dim
H = 8
WIN = 512        # sliding window (left 256, right 256)
ST = 256         # query supertile
NST = S // ST    # 16
KT = 128         # key tile
NKT = S // KT    # 32
N_CORES = 8
F32 = mybir.dt.float32
F32R = mybir.dt.float32r


def _round_fp32r(a: np.ndarray) -> np.ndarray:
    """Round fp32 array to the fp32r encoding (12-bit mantissa, round-half-up)."""
    u = np.ascontiguousarray(a, dtype=np.float32).view(np.uint32)
    u = (u + np.uint32(0x800)) & np.uint32(0xFFFFF000)
    return u.view(np.float32)


def _build_masks(ng: int):
    """Static 0/1 masks for the transposed [k=128, q=256] logit tiles.

    For supertile t and ktile j, delta = j - 2t and d = q - k =
    qq - kk + (-delta)*128 with qq in [0,256), kk in [0,128).
    Band keeps d in [-256, 255].
    delta=-2 -> keep qq <= kk - 1;   delta=-1 -> keep qq <= kk + 127
    delta=+2 -> keep qq >= kk;       delta=+3 -> keep qq >= kk + 128
    """
    kk = np.arange(KT)[:, None]
    qq = np.arange(ST)[None, :]
    m_m2 = (qq <= kk - 1).astype(np.float32)
    m_m1 = (qq <= kk + 127).astype(np.float32)
    m_p2 = (qq >= kk).astype(np.float32)
    m_p3 = (qq >= kk + 128).astype(np.float32)
    ml = np.concatenate([m_m2, m_m1], axis=1)            # [128, 512]
    mr = np.concatenate([m_p2, m_p3], axis=1)            # [128, 512]
    m_m2g = m_m2.copy()
    if ng > 0:
        m_m2g[:ng, :] = 1.0                              # global k rows always kept
    mlg = np.concatenate([m_m2g, m_m1], axis=1)          # used at t=1 (ktile 0)
    return ml, mr, mlg


def _sw_tiles(t: int):
    """ktile range and mask placements for supertile t."""
    j0 = max(0, 2 * t - 2)
    j1 = min(NKT, 2 * t + 4)
    # mask col offsets within the local psum layout (slice s holds ktile j0+s)
    ml_off = (2 * t - 2 - j0) * ST if 2 * t - 2 >= j0 else None  # always 0 when present
    ml_present = 2 * t - 2 >= 0
    mr_present = 2 * t + 2 < j1
    mr_off = (2 * t + 2 - j0) * ST if mr_present else None
    return j0, j1, ml_present, mr_off


def _build_program(ng: int):
    """Build + compile the per-core bass program, specialized for ng leading
    global tokens (0 <= ng <= 128)."""
    nc = bacc.Bacc("TRN2", target_bir_lowering=False, debug=False,
                   num_devices=N_CORES)

    d = {}
    d["xqT"] = nc.dram_tensor("xqT", [F, S], F32R, kind="ExternalInput").ap()
    d["xkvT"] = nc.dram_tensor("xkvT", [F, S], F32R, kind="ExternalInput").ap()
    for w in ("wq", "wk", "wv"):
        d[w] = nc.dram_tensor(w, [F, 2 * HD], F32R, kind="ExternalInput").ap()
    for b in ("bq", "bk", "bv"):
        d[b] = nc.dram_tensor(b, [2 * HD, 1], F32, kind="ExternalInput").ap()
    d["wo"] = nc.dram_tensor("wo", [HD, F], F32R, kind="ExternalInput").ap()
    d["ml"] = nc.dram_tensor("ml", [KT, 2 * ST], F32R, kind="ExternalInput").ap()
    d["mr"] = nc.dram_tensor("mr", [KT, 2 * ST], F32R, kind="ExternalInput").ap()
    d["mlg"] = nc.dram_tensor("mlg", [KT, 2 * ST], F32R, kind="ExternalInput").ap()
    d["ident"] = nc.dram_tensor("ident", [128, 128], F32, kind="ExternalInput").ap()
    out_ap = nc.dram_tensor("out", [S, F], F32, kind="ExternalOutput").ap()

    SC = 512            # projection s-chunk
    NSC = S // SC       # 8
    FT = F // 128       # 4 f-chunks

    with tile.TileContext(nc) as tc:
        with (
            tc.tile_pool(name="const", bufs=1) as constp,
            tc.tile_pool(name="big", bufs=1) as bigp,
        ):
            # ---- constants / persistent tensors ----
            wq_sb = constp.tile([128, FT, 128], F32R, tag="wq")
            wk_sb = constp.tile([128, FT, 128], F32R, tag="wk")
            wv_sb = constp.tile([128, FT, 128], F32R, tag="wv")
            for wsb, wap in ((wq_sb, d["wq"]), (wk_sb, d["wk"]), (wv_sb, d["wv"])):
                nc.sync.dma_start(wsb[:], wap.rearrange("(c p) e -> p c e", p=128))
            bq_sb = constp.tile([128, 1], F32, tag="bq")
            bk_sb = constp.tile([128, 1], F32, tag="bk")
            bv_sb = constp.tile([128, 1], F32, tag="bv")
            for bsb, bap in ((bq_sb, d["bq"]), (bk_sb, d["bk"]), (bv_sb, d["bv"])):
                nc.sync.dma_start(bsb[:], bap[:])
            wo_sb = constp.tile([HD, F], F32R, tag="wo")
            nc.sync.dma_start(wo_sb[:], d["wo"][:])
            ml_sb = constp.tile([KT, 2 * ST], F32R, tag="ml")
            mr_sb = constp.tile([KT, 2 * ST], F32R, tag="mr")
            mlg_sb = constp.tile([KT, 2 * ST], F32R, tag="mlg")
            nc.sync.dma_start(ml_sb[:], d["ml"][:])
            nc.sync.dma_start(mr_sb[:], d["mr"][:])
            if ng > 0:
                nc.sync.dma_start(mlg_sb[:], d["mlg"][:])
            id_sb = constp.tile([128, 128], F32, tag="id")
            nc.sync.dma_start(id_sb[:], d["ident"][:])
            ones32 = constp.tile([128, NKT], F32, tag="ones32")
            nc.vector.memset(ones32[:], 1.0)
            one_sb = constp.tile([128, 1], F32R, tag="one")
            nc.vector.tensor_copy(one_sb[:], ones32[:, 0:1])

            qT = bigp.tile([128, S], F32R, tag="qT")     # rows 0:64 sw, 64:128 g
            kT = bigp.tile([128, S], F32R, tag="kT")
            vsw = bigp.tile([128, NKT, HD + 1], F32R, tag="vsw")  # [s%128, kt, d|1]
            vg = bigp.tile([128, NKT, HD + 1], F32R, tag="vg")
            nc.vector.tensor_copy(vsw[:, :, HD], ones32[:])
            nc.vector.tensor_copy(vg[:, :, HD], ones32[:])

            # ================= Phase A: projections =================
            with (
                tc.tile_pool(name="xin", bufs=2) as xinp,
                tc.tile_pool(name="vtmp", bufs=2) as vtmpp,
                tc.tile_pool(name="pa", bufs=4, space="PSUM") as pap,
                tc.tile_pool(name="ptr", bufs=2, space="PSUM") as ptrp,
            ):
                for sc in range(NSC):
                    ss = sc * SC
                    xq_t = xinp.tile([128, FT, SC], F32R, tag="xq")
                    xkv_t = xinp.tile([128, FT, SC], F32R, tag="xkv")
                    nc.sync.dma_start(
                        xq_t[:], d["xqT"].rearrange("(c p) s -> p c s", p=128)[:, :, ss:ss + SC])
                    nc.sync.dma_start(
                        xkv_t[:], d["xkvT"].rearrange("(c p) s -> p c s", p=128)[:, :, ss:ss + SC])

                    pq = pap.tile([128, SC], F32, tag="pa")
                    for ft in range(FT):
                        nc.tensor.matmul(pq[:], wq_sb[:, ft, :], xq_t[:, ft, :],
                                         start=(ft == 0), stop=(ft == FT - 1))
                    nc.vector.tensor_scalar_add(qT[:, ss:ss + SC], pq[:], bq_sb[:, 0:1])

                    pk = pap.tile([128, SC], F32, tag="pa")
                    for ft in range(FT):
                        nc.tensor.matmul(pk[:], wk_sb[:, ft, :], xkv_t[:, ft, :],
                                         start=(ft == 0), stop=(ft == FT - 1))
                    nc.vector.tensor_scalar_add(kT[:, ss:ss + SC], pk[:], bk_sb[:, 0:1])

                    pv = pap.tile([128, SC], F32, tag="pa")
                    for ft in range(FT):
                        nc.tensor.matmul(pv[:], wv_sb[:, ft, :], xkv_t[:, ft, :],
                                         start=(ft == 0), stop=(ft == FT - 1))
                    vt_tmp = vtmpp.tile([128, SC], F32, tag="vt")
                    nc.vector.tensor_scalar_add(vt_tmp[:], pv[:], bv_sb[:, 0:1])
                    # transpose each 128-col block to natural [s, d] layout
                    for sb in range(SC // 128):
                        kt_idx = sc * (SC // 128) + sb
                        ptr = ptrp.tile([128, 128], F32, tag="tr")
                        nc.tensor.transpose(ptr[:], vt_tmp[:, sb * 128:(sb + 1) * 128], id_sb[:])
                        nc.vector.tensor_copy(vsw[:, kt_idx, 0:HD], ptr[:, 0:HD])
                        nc.vector.tensor_copy(vg[:, kt_idx, 0:HD], ptr[:, HD:2 * HD])

            # ================= Phase B: global attention (rows < ng) ============
            if ng > 0:
                with (
                    tc.tile_pool(name="eg", bufs=1) as egp,
                    tc.tile_pool(name="gx", bufs=1) as gxp,
                    tc.tile_pool(name="pb", bufs=4, space="PSUM") as pbp,
                    tc.tile_pool(name="pbs", bufs=1, space="PSUM") as pbsp,
                    tc.tile_pool(name="pbx", bufs=1, space="PSUM") as pbxp,
                    tc.tile_pool(name="pbo", bufs=1, space="PSUM") as pbop,
                ):
                    eg = egp.tile([128, NKT, ng], F32R, tag="eg")
                    for kt in range(NKT):
                        plg = pbp.tile([128, ng], F32, tag="lg")
                        nc.tensor.matmul(plg[:], kT[64:128, kt * KT:(kt + 1) * KT],
                                         qT[64:128, 0:ng], start=True, stop=True)
                        nc.scalar.activation(eg[:, kt, :], plg[:],
                                             mybir.ActivationFunctionType.Exp,
                                             scale=0.125)
                    pxg = pbxp.tile([HD + 1, ng], F32, tag="xg")
                    for kt in range(NKT):
                        nc.tensor.matmul(pxg[:], vg[:, kt, :], eg[:, kt, :],
                                         start=(kt == 0), stop=(kt == NKT - 1))
                    xgT = gxp.tile([HD + 1, ng], F32R, tag="xgT")
                    nc.vector.tensor_copy(xgT[:], pxg[:])
                    psg = pbsp.tile([ng, 1], F32, tag="sg")
                    nc.tensor.matmul(psg[:], xgT[HD:HD + 1, 0:ng].bitcast(F32),
                                     one_sb[HD:HD + 1, 0:1].bitcast(F32),
                                     start=True, stop=True)
                    rg = gxp.tile([ng, 1], F32, tag="rg")
                    nc.vector.reciprocal(rg[:], psg[:])
                    pog = pbop.tile([ng, F], F32, tag="og")
                    nc.tensor.matmul(pog[:], xgT[0:HD, 0:ng], wo_sb[:],
                                     start=True, stop=True)
                    og = gxp.tile([ng, F], F32, tag="og_sb")
                    nc.vector.tensor_scalar_mul(og[:], pog[:], rg[:, 0:1])
                    nc.sync.dma_start(out_ap[0:ng, :], og[:])

            # ================= Phase C: sliding-window attention ================
            with (
                tc.tile_pool(name="E", bufs=3) as ep,
                tc.tile_pool(name="xt", bufs=3) as xtp,
                tc.tile_pool(name="osb", bufs=3) as osbp,
                tc.tile_pool(name="rc", bufs=4) as rcp,
                tc.tile_pool(name="pL", bufs=3, space="PSUM") as pLp,
                tc.tile_pool(name="pX", bufs=2, space="PSUM") as pXp,
                tc.tile_pool(name="pS", bufs=1, space="PSUM") as pSp,
                tc.tile_pool(name="pO", bufs=2, space="PSUM") as pOp,
            ):
                for t in range(NST):
                    qs = t * ST
                    j0, j1, ml_present, mr_off = _sw_tiles(t)
                    nkt = j1 - j0
                    has_g = ng > 0 and j0 > 0
                    # 2-ktile groups: one psum bank each, finer PE<->ACT pipeline
                    E = ep.tile([128, 6 * ST], F32R, tag="E")
                    for a in range(0, nkt, 2):
                        b = min(a + 2, nkt)
                        pl = pLp.tile([128, (b - a) * ST], F32, tag="L")
                        for s in range(b - a):
                            j = j0 + a + s
                            nc.tensor.matmul(pl[:, s * ST:(s + 1) * ST],
                                             kT[0:64, j * KT:(j + 1) * KT],
                                             qT[0:64, qs:qs + ST],
                                             start=True, stop=True)
                        nc.scalar.activation(E[:, a * ST:b * ST], pl[:],
                                             mybir.ActivationFunctionType.Exp,
                                             scale=0.125)
                    # masks (ML on gpsimd, MR on vector to balance engines)
                    if ml_present:
                        msk = mlg_sb if (t == 1 and ng > 0) else ml_sb
                        nc.gpsimd.tensor_mul(E[:, 0:2 * ST], E[:, 0:2 * ST], msk[:])
                    if mr_off is not None:
                        nc.vector.tensor_mul(E[:, mr_off:mr_off + 2 * ST],
                                             E[:, mr_off:mr_off + 2 * ST], mr_sb[:])
                    if has_g:
                        plg2 = pLp.tile([ng, ST], F32, tag="L")
                        nc.tensor.matmul(plg2[:], kT[0:64, 0:ng], qT[0:64, qs:qs + ST],
                                         start=True, stop=True)
                        Eg2 = ep.tile([ng, ST], F32R, tag="Eg")
                        nc.scalar.activation(Eg2[:], plg2[:],
                                             mybir.ActivationFunctionType.Exp,
                                             scale=0.125)
                    # AV: xT' = [v|1].T @ expw.T  -> [65, 256], sums in row 64
                    px = pXp.tile([HD + 1, ST], F32, tag="X")
                    for s in range(nkt):
                        j = j0 + s
                        nc.tensor.matmul(px[:], vsw[:, j, :], E[:, s * ST:(s + 1) * ST],
                                         start=(s == 0),
                                         stop=(s == nkt - 1 and not has_g))
                    if has_g:
                        nc.tensor.matmul(px[:], vsw[0:ng, 0, :], Eg2[:],
                                         start=False, stop=True)
                    xT = xtp.tile([HD + 1, ST], F32R, tag="xT")
                    nc.vector.tensor_copy(xT[:], px[:])
                    for hf in range(ST // 128):
                        ps = pSp.tile([128, 1], F32, tag="S")
                        nc.tensor.matmul(ps[:],
                                         xT[HD:HD + 1, hf * 128:(hf + 1) * 128].bitcast(F32),
                                         one_sb[HD:HD + 1, 0:1].bitcast(F32),
                                         start=True, stop=True)
                        rc = rcp.tile([128, 1], F32, tag="rc")
                        nc.vector.reciprocal(rc[:], ps[:])
                        po = pOp.tile([128, F], F32, tag="O")
                        nc.tensor.matmul(po[:], xT[0:HD, hf * 128:(hf + 1) * 128],
                                         wo_sb[:], start=True, stop=True)
                        osb = osbp.tile([128, F], F32, tag="osb")
                        nc.vector.tensor_scalar_mul(osb[:], po[:], rc[:, 0:1])
                        r0 = qs + hf * 128
                        if r0 == 0 and ng > 0:
                            nc.sync.dma_start(out_ap[ng:128, :], osb[ng:128, :])
                        else:
                            nc.sync.dma_start(out_ap[r0:r0 + 128, :], osb[:])

    nc.compile()
    return nc


@functools.lru_cache(maxsize=4)
def _get_program(ng: int):
    return _build_program(ng)


def kernel(inputs_q, inputs_kv, global_mask,
           w_q_sw, b_q_sw, w_k_sw, b_k_sw, w_v_sw, b_v_sw,
           w_q_g, b_q_g, w_k_g, b_k_g, w_v_g, b_v_g,
           w_out, b_out,
           _trace=False, _tmpdir=None):
    gm = np.asarray(global_mask[0]).astype(bool)
    ng = int(gm.sum())
    assert gm[:ng].all() and not gm[ng:].any(), "global_mask must be a prefix mask"
    assert ng <= 128, "kernel specialized for ng <= 128"

    xqT = _round_fp32r(np.asarray(inputs_q[0], np.float32).T)
    xkvT = _round_fp32r(np.asarray(inputs_kv[0], np.float32).T)
    ml, mr, mlg = _build_masks(ng)
    ident = np.eye(128, dtype=np.float32)

    nc = _get_program(ng)

    in_maps = []
    for h in range(N_CORES):
        wq = _round_fp32r(np.concatenate([w_q_sw[:, h, :], w_q_g[:, h, :]], axis=1))
        wk = _round_fp32r(np.concatenate([w_k_sw[:, h, :], w_k_g[:, h, :]], axis=1))
        wv = _round_fp32r(np.concatenate([w_v_sw[:, h, :], w_v_g[:, h, :]], axis=1))
        bq = np.concatenate([b_q_sw[h], b_q_g[h]]).reshape(2 * HD, 1).astype(np.float32)
        bk = np.concatenate([b_k_sw[h], b_k_g[h]]).reshape(2 * HD, 1).astype(np.float32)
        bv = np.concatenate([b_v_sw[h], b_v_g[h]]).reshape(2 * HD, 1).astype(np.float32)
        wo = _round_fp32r(np.asarray(w_out[h], np.float32))
        in_maps.append({
            "xqT": xqT, "xkvT": xkvT,
            "wq": wq, "wk": wk, "wv": wv,
            "bq": bq, "bk": bk, "bv": bv,
            "wo": wo, "ml": ml, "mr": mr, "mlg": mlg, "ident": ident,
        })

    res = run_bass_kernel_spmd(nc, in_maps, list(range(N_CORES)),
                               trace=_trace, tmpdir=_tmpdir)
    partial = np.stack([res.results[h]["out"] for h in range(N_CORES)])
    out = partial.sum(axis=0) + np.asarray(b_out, np.float32)
    if _trace:
        kernel._last_results = res
    return out[None].astype(np.float32)

